# revision 21
# baseline (speedup 1.0000x reference)
"""Sparse KV block gather on 8 Trainium2 NeuronCores.

Problem: kv (32, 2, 64, 49, 256) f32 -> kv_flat (32, 128, 49*256);
out[b, q, k] = kv_flat[b, r_idx[b, q, k]]  -> (32, 64, 8, 49, 256).

Sharding: batch dim n=32 split across 8 cores (4 batches/core).

Strategy (v6, bf16 + fp8 one-hot, race-hardened): the rel-err gate
(2e-2) admits bf16 (max rounding error ~2^-9), so all HBM traffic is
halved: kv is shipped bf16 (12.85 MB read/core) and the gathered output
is written bf16 (51.4 MB/core) then up-cast to f32 on the host (exact:
the PSUM fp32 values are bf16 values).  The one-hot selection matrices
ship as fp8e4 (0.25 MB/core; 1.0 is exact in e4m3).

Each core stages its entire kv slice in SBUF (4 batches x 128 blocks,
one block per partition, 100 KB/partition).  The gather is a dynamic
partition permutation done on TensorE as matmuls against the one-hot
lhsT (bitwise exact: 1.0*x accumulated in fp32 PSUM).  PSUM is drained
(cast fp32->bf16) to SBUF in two-bank pairs by VectorE/ScalarE
alternately into 6 rotating stage slots, then written to HBM.  HBM
traffic/core: 63.9 MB at the measured ~380 GB/s -> ~168 us stream;
TensorE (448 matmuls, ~340 ns effective) runs concurrently.

Race hardening: a DMA's then_inc(sem, 16) is 16 independent per-SDMA-
engine +1s, and engines interleave queued DMAs, so waiting for 16*m on
a sem shared by several in-flight DMAs can pass with partial credits
from LATER transfers while an earlier one still has a laggard engine
in flight (observed once as zeroed output blocks).  Every wait here is
therefore exact: each load segment and oh half has a dedicated
semaphore (one DMA each), and out-DMA completion is tracked per stage
slot (s_sl[g%6]), where the drain-before-issue chain guarantees at
most one same-slot transfer in flight.

Queue layout (avoids FIFO head-of-line stalls and SWDGE-ucode issue
latency): kv loads ride scalar/ACT's otherwise-idle HWDGE ring;
sync/HWDGE carries oh, all even groups, odd groups 1,3 (so early
slot-reuse waits never sit behind backlog) and the final group's tail
pieces; gpsimd/SWDGE carries the remaining odd groups.
"""

from contextlib import ExitStack

import ml_dtypes
import numpy as np

import concourse.bacc as bacc
import concourse.bass as bass
import concourse.mybir as mybir
from concourse._compat import get_trn_type
from concourse.bass_utils import run_bass_kernel_spmd

# Problem shapes (hardcoded per contract: kernel.py is self-contained).
N, V, P2, W2, CKV = 32, 2, 64, 49, 256
TOPK = 8
NCORES = 8
NB = N // NCORES             # 4 batches per core
BLOCKS = V * P2              # 128 source blocks per batch
ELEM = W2 * CKV              # 12544 elems per block (25088 B bf16)
IDX_PER_B = P2 * TOPK        # 512 gathered blocks per batch
JCHUNK = 128                 # output blocks per one-hot matmul group
NJC = IDX_PER_B // JCHUNK    # 4 j-chunks per batch
FT = 448                     # f-columns per matmul tile (12544 = 28*448)
NFT = ELEM // FT             # 28 tiles per j-chunk
HALF = NFT // 2              # 14 tiles per DMA-out group (6272 elems)
NT = NB * NJC * NFT          # 448 matmul tiles per core
NG = NT // HALF              # 32 DMA-out groups per core
NPAIR = NT // 2              # 224 two-bank drain pairs
NSLOT = 6                    # rotating stage slots

BF16 = mybir.dt.bfloat16
FP8 = mybir.dt.float8e4

# kv load segments (k-tile ranges) per batch, one DMA + one semaphore
# each; batch 0 is finely split so matmuls start almost immediately.
SEGS = [(0, 0, 2), (0, 2, 7), (0, 7, 14), (0, 14, 21), (0, 21, 28),
        (1, 0, 28), (2, 0, 28), (3, 0, 28)]

# odd out-groups issued by sync rather than gpsimd: the early ones so
# their completions never wait on queue backlog, and the final group so
# its tail pieces issue at HWDGE (not SWDGE-ucode) latency.
SYNC_ODD = (1, 3, NG - 1)

_CACHE = {}


def _build_nc():
    nc = bacc.Bacc(get_trn_type() or "TRN2")
    kv_in = nc.dram_tensor("kv", [NB, BLOCKS, ELEM], BF16, kind="ExternalInput")
    oh_in = nc.dram_tensor(
        "oh", [128, NB * NJC * JCHUNK], FP8, kind="ExternalInput"
    )
    out = nc.dram_tensor(
        "out", [NB, NJC, JCHUNK, ELEM], BF16, kind="ExternalOutput"
    )

    with ExitStack() as stack:
        ec = stack.enter_context
        kv_sb = ec(nc.sbuf_tensor("kv_sb", [128, NB, ELEM], BF16))
        oh_sb = ec(nc.sbuf_tensor("oh_sb", [128, NB * NJC * JCHUNK], FP8))
        stage = ec(nc.sbuf_tensor("stage", [128, NSLOT, HALF * FT], BF16))
        ps = ec(nc.psum_tensor("ps", [128, 8, 512], mybir.dt.float32))
        s_oh1 = ec(nc.semaphore("s_oh1"))
        s_oh2 = ec(nc.semaphore("s_oh2"))
        s_ld = [ec(nc.semaphore(f"s_ld{i}")) for i in range(len(SEGS))]
        s_mm = ec(nc.semaphore("s_mm"))
        s_drv = ec(nc.semaphore("s_drv"))  # DVE drains (even pairs)
        s_dra = ec(nc.semaphore("s_dra"))  # ACT drains (odd pairs)
        # out DMA completion, per stage slot g%NSLOT
        s_sl = [ec(nc.semaphore(f"s_sl{i}")) for i in range(NSLOT)]
        block = ec(nc.Block())
        # tensor waits s_ld[seg] >= 16 before first use of that range
        seg_of = {(n, k0): i for i, (n, k0, k1) in enumerate(SEGS)}

        def group_pieces(g):
            t0 = g * HALF
            n = t0 // (NJC * NFT)
            c = (t0 // NFT) % NJC
            h = (t0 % NFT) // HALF
            f0 = h * HALF * FT
            # final group: smaller DMAs to shorten the tail
            pieces = (
                [(0, HALF)]
                if g < NG - 1
                else [(0, 7), (7, 11), (11, 13), (13, HALF)]
            )
            return [(n, c, f0, p0, p1, t0) for p0, p1 in pieces]

        def issue_group(eng, g):
            for n, c, f0, p0, p1, t0 in group_pieces(g):
                # drain pairs [0, ceil((t0+p1)/2)) must have completed
                # (drains are pair-atomic, so odd boundaries round up)
                P1 = (t0 + p1 + 1) // 2
                eng.wait_ge(s_drv, (P1 + 1) // 2)
                eng.wait_ge(s_dra, P1 // 2)
                eng.dma_start(
                    out=out[n, c, :, f0 + p0 * FT : f0 + p1 * FT],
                    in_=stage[:, g % NSLOT, p0 * FT : p1 * FT],
                ).then_inc(s_sl[g % NSLOT], 16)

        # total per-slot out-DMA piece counts, for the final wait
        slot_total = [0] * NSLOT
        for g in range(NG):
            slot_total[g % NSLOT] += len(group_pieces(g))

        @block.gpsimd
        def _(gpsimd):
            # the odd out-groups not taken by sync
            for g in range(1, NG, 2):
                if g not in SYNC_ODD:
                    issue_group(gpsimd, g)

        @block.tensor
        def _(tensor):
            tensor.wait_ge(s_oh1, 16)
            for t in range(NT):
                n = t // (NJC * NFT)
                c = (t // NFT) % NJC
                k = t % NFT
                if t == NFT:
                    # one-hots beyond the first j-chunk arrive in load 2
                    tensor.wait_ge(s_oh2, 16)
                if c == 0 and (n, k) in seg_of:
                    tensor.wait_ge(s_ld[seg_of[(n, k)]], 16)
                if t >= 8:
                    # PSUM banks (t-8, partner) free once pair (t-8)//2 drained
                    p = (t - 8) // 2
                    if p % 2 == 0:
                        tensor.wait_ge(s_drv, p // 2 + 1)
                    else:
                        tensor.wait_ge(s_dra, p // 2 + 1)
                tensor.matmul(
                    ps[:, t % 8, 0:FT],
                    oh_sb[:, (n * NJC + c) * JCHUNK : (n * NJC + c + 1) * JCHUNK],
                    kv_sb[:, n, k * FT : (k + 1) * FT],
                    start=True,
                    stop=True,
                ).then_inc(s_mm, 1)

        def _drain(eng, parity, sem):
            # each drain copies a PAIR of consecutive PSUM banks (2 tiles)
            for p in range(parity, NPAIR, 2):
                t = 2 * p
                g = t // HALF
                kk = t % HALF
                b0 = t % 8
                eng.wait_ge(s_mm, t + 2)
                if g >= NSLOT:
                    # stage slot g%NSLOT free once group g-NSLOT's out-DMA
                    # (the g//NSLOT-th same-slot transfer) fully completed
                    eng.wait_ge(s_sl[g % NSLOT], 16 * (g // NSLOT))
                eng_copy = eng.tensor_copy if parity == 0 else eng.copy
                eng_copy(
                    stage[:, g % NSLOT, kk * FT : (kk + 2) * FT],
                    ps[:, b0 : b0 + 2, 0:FT],
                ).then_inc(sem, 1)

        @block.vector
        def _(vector):
            _drain(vector, 0, s_drv)

        @block.scalar
        def _(scalar):
            # kv loads ride ACT's otherwise-idle HWDGE ring: first bytes at
            # HWDGE (not SWDGE-ucode) latency, and no queue shares them.
            # kv is fully resident in SBUF so there are no reuse waits.
            for i, (n, k0, k1) in enumerate(SEGS):
                scalar.dma_start(
                    out=kv_sb[:, n, k0 * FT : k1 * FT],
                    in_=kv_in[n][:, k0 * FT : k1 * FT],
                ).then_inc(s_ld[i], 16)
            _drain(scalar, 1, s_dra)

        @block.sync
        def _(sync):
            # first j-chunk's one-hot first (16 KB) so matmuls start early
            sync.dma_start(
                out=oh_sb[:, 0:JCHUNK], in_=oh_in[:, 0:JCHUNK]
            ).then_inc(s_oh1, 16)
            sync.dma_start(
                out=oh_sb[:, JCHUNK:], in_=oh_in[:, JCHUNK:]
            ).then_inc(s_oh2, 16)
            order = sorted(list(range(0, NG, 2)) + list(SYNC_ODD))
            for g in order:
                issue_group(sync, g)
            for s in range(NSLOT):
                sync.wait_ge(s_sl[s], 16 * slot_total[s])

    nc.compile()
    return nc


def _prep_onehot(r_idx_core: np.ndarray) -> np.ndarray:
    """r_idx_core: (NB, P2, TOPK) -> one-hot lhsT in SBUF layout
    (128, NB*NJC*JCHUNK) fp8e4:  arr[i, g*128 + j] = 1 iff r_idx_flat[g, j] == i.
    """
    fp8 = mybir.dt.np(FP8)
    idx = r_idx_core.reshape(NB * NJC, JCHUNK).astype(np.int64)
    oh = np.zeros((NB * NJC, 128, JCHUNK), fp8)
    g = np.arange(NB * NJC)[:, None]
    j = np.arange(JCHUNK)[None, :]
    oh[g, idx, j] = 1.0
    return np.ascontiguousarray(oh.transpose(1, 0, 2).reshape(128, NB * NJC * JCHUNK))


def make_in_maps(r_idx: np.ndarray, kv: np.ndarray) -> list:
    kv_r = np.ascontiguousarray(
        np.asarray(kv).reshape(N, BLOCKS, ELEM).astype(ml_dtypes.bfloat16)
    )
    in_maps = []
    for c in range(NCORES):
        lo = c * NB
        in_maps.append(
            {
                "kv": kv_r[lo : lo + NB],
                "oh": _prep_onehot(np.asarray(r_idx)[lo : lo + NB]),
            }
        )
    return in_maps


def kernel(r_idx: np.ndarray, r_weight: np.ndarray, kv: np.ndarray) -> np.ndarray:
    if "nc" not in _CACHE:
        _CACHE["nc"] = _build_nc()
    nc = _CACHE["nc"]

    in_maps = make_in_maps(r_idx, kv)
    res = run_bass_kernel_spmd(nc, in_maps, core_ids=list(range(NCORES)))
    outs = [
        res.results[c]["out"]
        .reshape(NB, P2, TOPK, W2, CKV)
        .astype(np.float32)
        for c in range(NCORES)
    ]
    return np.concatenate(outs, axis=0)


# revision 24
# speedup vs baseline: 1.0015x; 1.0015x over previous
"""Sparse KV block gather on 8 Trainium2 NeuronCores.

Problem: kv (32, 2, 64, 49, 256) f32 -> kv_flat (32, 128, 49*256);
out[b, q, k] = kv_flat[b, r_idx[b, q, k]]  -> (32, 64, 8, 49, 256).

Sharding: batch dim n=32 split across 8 cores (4 batches/core).

Strategy (v6, bf16 + fp8 one-hot, race-hardened): the rel-err gate
(2e-2) admits bf16 (max rounding error ~2^-9), so all HBM traffic is
halved: kv is shipped bf16 (12.85 MB read/core) and the gathered output
is written bf16 (51.4 MB/core) then up-cast to f32 on the host (exact:
the PSUM fp32 values are bf16 values).  The one-hot selection matrices
ship as fp8e4 (0.25 MB/core; 1.0 is exact in e4m3).

Each core stages its entire kv slice in SBUF (4 batches x 128 blocks,
one block per partition, 100 KB/partition).  The gather is a dynamic
partition permutation done on TensorE as matmuls against the one-hot
lhsT (bitwise exact: 1.0*x accumulated in fp32 PSUM).  PSUM is drained
(cast fp32->bf16) to SBUF in two-bank pairs by VectorE/ScalarE
alternately into 6 rotating stage slots, then written to HBM.  HBM
traffic/core: 63.9 MB at the measured ~380 GB/s -> ~168 us stream;
TensorE (448 matmuls, ~340 ns effective) runs concurrently.

Race hardening: a DMA's then_inc(sem, 16) is 16 independent per-SDMA-
engine +1s, and engines interleave queued DMAs, so waiting for 16*m on
a sem shared by several in-flight DMAs can pass with partial credits
from LATER transfers while an earlier one still has a laggard engine
in flight (observed once as zeroed output blocks).  Every wait here is
therefore exact: each load segment and oh half has a dedicated
semaphore (one DMA each), and out-DMA completion is tracked per stage
slot (s_sl[g%6]), where the drain-before-issue chain guarantees at
most one same-slot transfer in flight.

Queue layout (avoids FIFO head-of-line stalls): gpsimd/SWDGE carries
the kv loads then odd out-groups >=5; sync/HWDGE carries oh, all even
groups, odd groups 1,3 (so early slot-reuse waits never sit behind the
12.85 MB load burst) and the final group's four tail pieces (issued at
HWDGE, not SWDGE-ucode, latency).
"""

from contextlib import ExitStack

import ml_dtypes
import numpy as np

import concourse.bacc as bacc
import concourse.bass as bass
import concourse.mybir as mybir
from concourse._compat import get_trn_type
from concourse.bass_utils import run_bass_kernel_spmd

# Problem shapes (hardcoded per contract: kernel.py is self-contained).
N, V, P2, W2, CKV = 32, 2, 64, 49, 256
TOPK = 8
NCORES = 8
NB = N // NCORES             # 4 batches per core
BLOCKS = V * P2              # 128 source blocks per batch
ELEM = W2 * CKV              # 12544 elems per block (25088 B bf16)
IDX_PER_B = P2 * TOPK        # 512 gathered blocks per batch
JCHUNK = 128                 # output blocks per one-hot matmul group
NJC = IDX_PER_B // JCHUNK    # 4 j-chunks per batch
FT = 448                     # f-columns per matmul tile (12544 = 28*448)
NFT = ELEM // FT             # 28 tiles per j-chunk
HALF = NFT // 2              # 14 tiles per DMA-out group (6272 elems)
NT = NB * NJC * NFT          # 448 matmul tiles per core
NG = NT // HALF              # 32 DMA-out groups per core
NPAIR = NT // 2              # 224 two-bank drain pairs
NSLOT = 6                    # rotating stage slots

BF16 = mybir.dt.bfloat16
FP8 = mybir.dt.float8e4

# kv load segments (k-tile ranges) per batch, one DMA + one semaphore
# each; batch 0 is finely split so matmuls start almost immediately.
SEGS = [(0, 0, 2), (0, 2, 7), (0, 7, 14), (0, 14, 21), (0, 21, 28),
        (1, 0, 28), (2, 0, 28), (3, 0, 28)]

# odd out-groups issued by sync rather than gpsimd: the early ones so
# their completions never wait on queue backlog, and the final group so
# its tail pieces issue at HWDGE (not SWDGE-ucode) latency.
SYNC_ODD = (1, 3, NG - 1)

_CACHE = {}


def _build_nc():
    nc = bacc.Bacc(get_trn_type() or "TRN2")
    kv_in = nc.dram_tensor("kv", [NB, BLOCKS, ELEM], BF16, kind="ExternalInput")
    oh_in = nc.dram_tensor(
        "oh", [128, NB * NJC * JCHUNK], FP8, kind="ExternalInput"
    )
    out = nc.dram_tensor(
        "out", [NB, NJC, JCHUNK, ELEM], BF16, kind="ExternalOutput"
    )

    with ExitStack() as stack:
        ec = stack.enter_context
        kv_sb = ec(nc.sbuf_tensor("kv_sb", [128, NB, ELEM], BF16))
        oh_sb = ec(nc.sbuf_tensor("oh_sb", [128, NB * NJC * JCHUNK], FP8))
        stage = ec(nc.sbuf_tensor("stage", [128, NSLOT, HALF * FT], BF16))
        ps = ec(nc.psum_tensor("ps", [128, 8, 512], mybir.dt.float32))
        s_oh1 = ec(nc.semaphore("s_oh1"))
        s_oh2 = ec(nc.semaphore("s_oh2"))
        s_ld = [ec(nc.semaphore(f"s_ld{i}")) for i in range(len(SEGS))]
        s_mm = ec(nc.semaphore("s_mm"))
        s_drv = ec(nc.semaphore("s_drv"))  # DVE drains (even pairs)
        s_dra = ec(nc.semaphore("s_dra"))  # ACT drains (odd pairs)
        # out DMA completion, per stage slot g%NSLOT
        s_sl = [ec(nc.semaphore(f"s_sl{i}")) for i in range(NSLOT)]
        block = ec(nc.Block())
        # tensor waits s_ld[seg] >= 16 before first use of that range
        seg_of = {(n, k0): i for i, (n, k0, k1) in enumerate(SEGS)}

        def group_pieces(g):
            t0 = g * HALF
            n = t0 // (NJC * NFT)
            c = (t0 // NFT) % NJC
            h = (t0 % NFT) // HALF
            f0 = h * HALF * FT
            # final group: smaller DMAs to shorten the tail
            pieces = (
                [(0, HALF)]
                if g < NG - 1
                else [(0, 7), (7, 11), (11, 13), (13, HALF)]
            )
            return [(n, c, f0, p0, p1, t0) for p0, p1 in pieces]

        def issue_group(eng, g):
            for n, c, f0, p0, p1, t0 in group_pieces(g):
                # drain pairs [0, ceil((t0+p1)/2)) must have completed
                # (drains are pair-atomic, so odd boundaries round up)
                P1 = (t0 + p1 + 1) // 2
                eng.wait_ge(s_drv, (P1 + 1) // 2)
                eng.wait_ge(s_dra, P1 // 2)
                eng.dma_start(
                    out=out[n, c, :, f0 + p0 * FT : f0 + p1 * FT],
                    in_=stage[:, g % NSLOT, p0 * FT : p1 * FT],
                ).then_inc(s_sl[g % NSLOT], 16)

        # total per-slot out-DMA piece counts, for the final wait
        slot_total = [0] * NSLOT
        for g in range(NG):
            slot_total[g % NSLOT] += len(group_pieces(g))

        @block.gpsimd
        def _(gpsimd):
            # kv fully resident: no reuse waits, just stream the loads in.
            # (SWDGE issue here is off every critical path: HWDGE dma_start
            # costs ~0.6us of sequencer time, which scalar/sync can't spare
            # early, while gpsimd is otherwise idle.)
            for i, (n, k0, k1) in enumerate(SEGS):
                gpsimd.dma_start(
                    out=kv_sb[:, n, k0 * FT : k1 * FT],
                    in_=kv_in[n][:, k0 * FT : k1 * FT],
                ).then_inc(s_ld[i], 16)
            # then the odd out-groups not taken by sync
            for g in range(1, NG, 2):
                if g not in SYNC_ODD:
                    issue_group(gpsimd, g)

        @block.tensor
        def _(tensor):
            tensor.wait_ge(s_oh1, 16)
            for t in range(NT):
                n = t // (NJC * NFT)
                c = (t // NFT) % NJC
                k = t % NFT
                if t == NFT:
                    # one-hots beyond the first j-chunk arrive in load 2
                    tensor.wait_ge(s_oh2, 16)
                if c == 0 and (n, k) in seg_of:
                    tensor.wait_ge(s_ld[seg_of[(n, k)]], 16)
                if t >= 8:
                    # PSUM banks (t-8, partner) free once pair (t-8)//2 drained
                    p = (t - 8) // 2
                    if p % 2 == 0:
                        tensor.wait_ge(s_drv, p // 2 + 1)
                    else:
                        tensor.wait_ge(s_dra, p // 2 + 1)
                tensor.matmul(
                    ps[:, t % 8, 0:FT],
                    oh_sb[:, (n * NJC + c) * JCHUNK : (n * NJC + c + 1) * JCHUNK],
                    kv_sb[:, n, k * FT : (k + 1) * FT],
                    start=True,
                    stop=True,
                ).then_inc(s_mm, 1)

        def _drain(eng, parity, sem):
            # each drain copies a PAIR of consecutive PSUM banks (2 tiles)
            for p in range(parity, NPAIR, 2):
                t = 2 * p
                g = t // HALF
                kk = t % HALF
                b0 = t % 8
                eng.wait_ge(s_mm, t + 2)
                if g >= NSLOT:
                    # stage slot g%NSLOT free once group g-NSLOT's out-DMA
                    # (the g//NSLOT-th same-slot transfer) fully completed
                    eng.wait_ge(s_sl[g % NSLOT], 16 * (g // NSLOT))
                eng_copy = eng.tensor_copy if parity == 0 else eng.copy
                eng_copy(
                    stage[:, g % NSLOT, kk * FT : (kk + 2) * FT],
                    ps[:, b0 : b0 + 2, 0:FT],
                ).then_inc(sem, 1)

        @block.vector
        def _(vector):
            _drain(vector, 0, s_drv)

        @block.scalar
        def _(scalar):
            _drain(scalar, 1, s_dra)

        @block.sync
        def _(sync):
            # first j-chunk's one-hot first (16 KB) so matmuls start early
            sync.dma_start(
                out=oh_sb[:, 0:JCHUNK], in_=oh_in[:, 0:JCHUNK]
            ).then_inc(s_oh1, 16)
            sync.dma_start(
                out=oh_sb[:, JCHUNK:], in_=oh_in[:, JCHUNK:]
            ).then_inc(s_oh2, 16)
            order = sorted(list(range(0, NG, 2)) + list(SYNC_ODD))
            for g in order:
                issue_group(sync, g)
            for s in range(NSLOT):
                sync.wait_ge(s_sl[s], 16 * slot_total[s])

    nc.compile()
    return nc


def _prep_onehot(r_idx_core: np.ndarray) -> np.ndarray:
    """r_idx_core: (NB, P2, TOPK) -> one-hot lhsT in SBUF layout
    (128, NB*NJC*JCHUNK) fp8e4:  arr[i, g*128 + j] = 1 iff r_idx_flat[g, j] == i.
    """
    fp8 = mybir.dt.np(FP8)
    idx = r_idx_core.reshape(NB * NJC, JCHUNK).astype(np.int64)
    oh = np.zeros((NB * NJC, 128, JCHUNK), fp8)
    g = np.arange(NB * NJC)[:, None]
    j = np.arange(JCHUNK)[None, :]
    oh[g, idx, j] = 1.0
    return np.ascontiguousarray(oh.transpose(1, 0, 2).reshape(128, NB * NJC * JCHUNK))


def make_in_maps(r_idx: np.ndarray, kv: np.ndarray) -> list:
    kv_r = np.ascontiguousarray(
        np.asarray(kv).reshape(N, BLOCKS, ELEM).astype(ml_dtypes.bfloat16)
    )
    in_maps = []
    for c in range(NCORES):
        lo = c * NB
        in_maps.append(
            {
                "kv": kv_r[lo : lo + NB],
                "oh": _prep_onehot(np.asarray(r_idx)[lo : lo + NB]),
            }
        )
    return in_maps


def kernel(r_idx: np.ndarray, r_weight: np.ndarray, kv: np.ndarray) -> np.ndarray:
    if "nc" not in _CACHE:
        _CACHE["nc"] = _build_nc()
    nc = _CACHE["nc"]

    in_maps = make_in_maps(r_idx, kv)
    res = run_bass_kernel_spmd(nc, in_maps, core_ids=list(range(NCORES)))
    outs = [
        res.results[c]["out"]
        .reshape(NB, P2, TOPK, W2, CKV)
        .astype(np.float32)
        for c in range(NCORES)
    ]
    return np.concatenate(outs, axis=0)
